# revision 9
# baseline (speedup 1.0000x reference)
"""Causal attention kernel for Trainium2, SPMD over 8 NeuronCores.

Problem (hardcoded): embeddings [4, 2048, 1024] f32, Wq/Wk/Wv [1024, 1024] f32.
    q = X Wq; k = X Wk; v = X Wv
    out = softmax(causal(q k^T) / 32) v          (per batch)

Sharding: 8 cores = (4 batches) x (2 q-shards). Each core handles 1024 query
rows of one batch as eight 128-row q-tiles with balanced causal work:
core parity 0 gets the even global q-tiles [0,2,..,14], parity 1 the odd ones.
Both see the same per-slot k-extent pattern [1..8] (in 256-wide k-slices) and
a single causal-mask pattern (offset 0 or 128), so one SPMD program serves
all 8 cores; all per-core divergence is carried by input data.

Per-call host<->device traffic over the axon tunnel is the dominant cost
(~38 MB/s up, ~29 MB/s down, transfers serialized), so the kernel ships
every input byte exactly once at the smallest dtype that holds the 2e-2
error gate (host-simulated rel err for this config: 1.10e-2):
  - xp [1024, 1280] u8: the core's OWN q-tile columns of X^T (ascending tile
    order), int10-packed (per 512-col group: 512 int8 high bytes = x10>>2,
    then 128 bytes of 2-bit low fields, byte j = lo[j] | lo[j+128]<<2 |
    lo[j+256]<<4 | lo[j+384]<<6) with a global power-of-2 scale so the
    device unpack rounds only once in bf16. Used directly as Xq^T, AND
    pair-AllGathered in halves: the two rank blocks (even tiles | odd
    tiles) interleave back into the full X^T in global key order.
    1.25 MiB/core.
  - wmsh [128, 1024] int8 / wvsh [128, 1024] int8: the core's 1/8 row-shards
    of wm = Wq @ Wk.T and Wv (both per-row int8-quantized on host); each
    all-8 AllGathered on device and dequantized to bf16 in SBUF.
  - thr [128, 20] f32: col 0 parity threshold for the device-generated
    causal mask; cols 1..8 wm dequant scales; col 9 the X scale s;
    cols 10..17 wv dequant scales; col 18 = 4s; col 19 = -4s.
Output downloads as int7 (8 values bit-packed into 7 bytes, [8, 128, 896] u8)
with per-row f32 scales (0.875 MiB/core), dequantized on host. Host-simulated
rel err for the full config: 1.45e-2 vs the 2e-2 gate.

The runner bypasses run_bass_kernel_spmd: it jits the bass_exec shard_map
body directly so the output donation buffers can be the PREVIOUS call's
device-resident outputs (run_bass_kernel_spmd uploads 8.4 MB of host zeros
per call just to donate them), and fetches outputs with copy_to_host_async.

Device math (same algebraic structure as the fp32 reference):
    G^T = wm^T Xq^T; S = G X^T (slabwise, causal-masked); P = exp(S/32+mask)
    unnormalized with row-sums via activation accumulate; V = X Wv;
    O = (P V) * 1/rowsum.  All matmuls bf16 with fp32 PSUM accumulation.
"""

import numpy as np

B = 4
S = 2048
E = 1024
D = 1024
P = 128
NCORES = 8
KSL = 512  # k-slice width

XW = 1280  # int10-packed X row bytes (2 groups of 512+128)
GB = 640  # bytes per 512-col group

TILES = [
    [0, 2, 4, 6, 8, 10, 12, 14],
    [1, 3, 5, 7, 9, 11, 13, 15],
]
CNT = [1, 2, 3, 4, 5, 6, 7, 8]  # 256-wide k-slices per slot
KA = 256  # causal-mask tile width

MASK_VAL = -1.0e30

_CACHE = {}


def _build_program():
    import concourse.bacc as bacc
    import concourse.tile as tile
    from concourse import mybir
    from concourse.masks import make_identity

    bf16 = mybir.dt.bfloat16
    f32 = mybir.dt.float32

    nc = bacc.Bacc("TRN2", target_bir_lowering=False, debug=False, num_devices=NCORES)

    i8 = mybir.dt.int8
    u8 = mybir.dt.uint8
    # weights ship as int8 row-shards (per-row scales in thr); AllGathered.
    # Declared BEFORE xp so they upload first: the all-8 weight gathers and
    # the wm/wv dequant then overlap the 10.5 MB xp upload.
    wmsh_d = nc.dram_tensor("wmsh", [E // NCORES, D], i8, kind="ExternalInput")
    wvsh_d = nc.dram_tensor("wvsh", [E // NCORES, D], i8, kind="ExternalInput")
    # X ships int10: per 512-col group, bytes [g*640, g*640+512) are the int8
    # high parts (x10 >> 2), bytes [g*640+512, (g+1)*640) the packed 2-bit
    # low fields (byte j = lo[j] | lo[j+128]<<2 | lo[j+256]<<4 | lo[j+384]<<6).
    # Global power-of-2 scale (thr col 9) -> unpack rounds once in bf16.
    xp_d = nc.dram_tensor("xp", [E, XW], u8, kind="ExternalInput")
    # col 0: parity threshold for the causal mask; cols 1..8: wm per-row
    # dequant scales (row co*128+ci -> [ci, 1+co]); col 9: x scale s;
    # cols 10..17: wv scales; col 18: 4s; col 19: -4s
    thr_d = nc.dram_tensor("thr", [P, 20], f32, kind="ExternalInput")
    # int7 egress (8 values bit-packed into 7 bytes) with per-row dynamic
    # scale: 7/8 the download bytes of int8
    out_d = nc.dram_tensor("out", [8, P, (D // 8) * 7], u8, kind="ExternalOutput")
    scl_d = nc.dram_tensor("scl", [8, P, 1], f32, kind="ExternalOutput")

    EO = E // P  # 8 e-chunks
    KT = S // P  # 16 k-tiles
    PAIRS = [[0, 1], [2, 3], [4, 5], [6, 7]]
    ALL8 = [list(range(NCORES))]

    with tile.TileContext(nc) as tc:
        with (
            tc.tile_pool(name="dram", bufs=1, space="DRAM") as dram,
            tc.tile_pool(name="persist", bufs=1) as persist,
            tc.tile_pool(name="big", bufs=1) as big,
            tc.tile_pool(name="psS", bufs=3, space="PSUM") as psS,
            tc.tile_pool(name="psT", bufs=3, space="PSUM") as psT,
            tc.tile_pool(name="psO", bufs=2, space="PSUM") as psO,
            tc.tile_pool(name="xup", bufs=1) as xup,
        ):
            # --- bounce + collectives (weights first: G^T unblocks on it;
            # X pair-gather split in column halves so the first half's V +
            # attention slots 0-3 hide under the second half's gather) ------
            xq_bnc_a = dram.tile([E, GB], u8)
            xq_bnc_b = dram.tile([E, GB], u8)
            wm_bnc = dram.tile([E // NCORES, D], i8)
            wv_bnc = dram.tile([E // NCORES, D], i8)
            xga = dram.tile([2, E, GB], u8)
            xgb = dram.tile([2, E, GB], u8)
            wm8g = dram.tile([E, D], i8, addr_space="Shared")
            wvg = dram.tile([E, D], i8, addr_space="Shared")
            nc.gpsimd.dma_start(wm_bnc[:], wmsh_d[:])
            nc.gpsimd.dma_start(wv_bnc[:], wvsh_d[:])
            nc.gpsimd.dma_start(xq_bnc_a[:], xp_d[:, 0:GB])
            nc.gpsimd.dma_start(xq_bnc_b[:], xp_d[:, GB : 2 * GB])
            nc.gpsimd.collective_compute(
                "AllGather",
                mybir.AluOpType.bypass,
                replica_groups=ALL8,
                ins=[wm_bnc.opt()],
                outs=[wm8g.opt()],
            )
            nc.gpsimd.collective_compute(
                "AllGather",
                mybir.AluOpType.bypass,
                replica_groups=PAIRS,
                ins=[xq_bnc_a.opt()],
                outs=[xga.opt()],
            )
            nc.gpsimd.collective_compute(
                "AllGather",
                mybir.AluOpType.bypass,
                replica_groups=ALL8,
                ins=[wv_bnc.opt()],
                outs=[wvg.opt()],
            )
            nc.gpsimd.collective_compute(
                "AllGather",
                mybir.AluOpType.bypass,
                replica_groups=PAIRS,
                ins=[xq_bnc_b.opt()],
                outs=[xgb.opt()],
            )

            # --- SBUF residents -------------------------------------------
            ident = persist.tile([P, P], bf16, tag="ident")
            make_identity(nc, ident)
            # causal mask built on device: kill when col - row > 128*parity;
            # the parity rides in as a tiny [P, 1] threshold upload
            masks_sb = persist.tile([P, KA], f32, tag="masks")
            iota_sb = persist.tile([P, KA], f32, tag="iota")
            thr_sb = persist.tile([P, 20], f32, tag="thr")
            nc.sync.dma_start(thr_sb, thr_d[:])
            nc.gpsimd.iota(
                iota_sb,
                pattern=[[1, KA]],
                base=0,
                channel_multiplier=-1,
                allow_small_or_imprecise_dtypes=True,
            )
            nc.vector.tensor_scalar(
                masks_sb,
                iota_sb,
                thr_sb[:, 0:1],
                MASK_VAL,
                mybir.AluOpType.is_gt,
                mybir.AluOpType.mult,
            )

            xq_sb = persist.tile([P, EO, P * 8], bf16, tag="xq")  # Xq^T [e, q]
            wm_sb = persist.tile([P, EO, D], bf16, tag="wm")  # wm [e, e']
            wv_sb = persist.tile([P, EO, D], bf16, tag="wv")  # Wv [e, d]
            gt = persist.tile([P, EO, P * 8], bf16, tag="gt")  # G^T [e', q]
            xt = big.tile([P, EO, S], bf16, tag="xt")  # X^T [e, s]
            v = big.tile([P, KT, D], bf16, tag="v")  # V [k, d]

            # --- int10 X unpack: dst strip i gets hi_i*4s + lo_i*s where
            # lo_i = (b>>2i) - 4*(b>>2(i+2? no: 2(i+1))). Combine the lo
            # terms first (lo_i*s is exact in bf16, 2 bits) then add the
            # exact hi_i*4s, so the sum rounds only once — matching the
            # host-side error simulation.
            xscl = thr_sb[:, 9:10]  # s
            s4 = thr_sb[:, 18:19]  # 4s
            sm4 = thr_sb[:, 19:20]  # -4s

            def unpack_x(hi_src, lo_src, dst_strips, name):
                hi_sb = xup.tile([P, EO, KSL], i8, tag="uhi", name=f"uhi_{name}")
                lo_sb = xup.tile([P, EO, P], u8, tag="ulo", name=f"ulo_{name}")
                t_sh = [
                    xup.tile([P, EO, P], u8, tag=f"ut{k}", name=f"ut{k}_{name}")
                    for k in range(1, 4)
                ]
                tmp = xup.tile([P, EO, P], bf16, tag="utmp", name=f"utmp_{name}")
                u_sb = xup.tile([P, EO, P], bf16, tag="uu", name=f"uu_{name}")
                nc.sync.dma_start(hi_sb, hi_src)
                nc.scalar.dma_start(lo_sb, lo_src)
                ts = [lo_sb] + t_sh
                for k in range(1, 4):
                    nc.vector.tensor_scalar(
                        ts[k], lo_sb, 2 * k, None, mybir.AluOpType.logical_shift_right
                    )
                for i in range(4):
                    dst = dst_strips[i]
                    if i < 3:
                        nc.vector.tensor_scalar_mul(tmp, ts[i + 1], sm4)
                        nc.vector.scalar_tensor_tensor(
                            u_sb,
                            ts[i],
                            xscl,
                            tmp,
                            mybir.AluOpType.mult,
                            mybir.AluOpType.add,
                        )
                    else:
                        nc.vector.tensor_scalar_mul(u_sb, ts[3], xscl)
                    nc.vector.scalar_tensor_tensor(
                        dst,
                        hi_sb[:, :, i * P : (i + 1) * P],
                        s4,
                        u_sb,
                        mybir.AluOpType.mult,
                        mybir.AluOpType.add,
                    )

            # my own q columns: unpack straight from my upload
            xp_r = xp_d.rearrange("(eo ei) b -> ei eo b", ei=P)
            for h in range(2):
                unpack_x(
                    xp_r[:, :, h * GB : h * GB + KSL].bitcast(i8),
                    xp_r[:, :, h * GB + KSL : (h + 1) * GB],
                    [
                        xq_sb[:, :, h * KSL + i * P : h * KSL + (i + 1) * P]
                        for i in range(4)
                    ],
                    f"xq{h}",
                )

            # wm int8 from the first all-8 gather (rank block r = wm rows
            # r*128..(r+1)*128, i.e. co=r, ci=partition), dequantized to bf16
            # with the per-row scales from thr cols 1..8; wv int8 likewise
            # from the second gather with scales from cols 10..17
            wm8_r = wm8g.rearrange("(co ci) e -> ci co e", ci=P)
            wm_i8_sb = persist.tile([P, EO, D], i8, tag="wm8")
            nc.sync.dma_start(wm_i8_sb, wm8_r)
            for co in range(EO):
                nc.vector.tensor_scalar_mul(
                    wm_sb[:, co, :], wm_i8_sb[:, co, :], thr_sb[:, 1 + co : 2 + co]
                )
            wv_r = wvg.rearrange("(co ci) d -> ci co d", ci=P)
            wv_i8_sb = persist.tile([P, EO, D], i8, tag="wv8")
            nc.scalar.dma_start(wv_i8_sb, wv_r)
            for co in range(EO):
                nc.vector.tensor_scalar_mul(
                    wv_sb[:, co, :], wv_i8_sb[:, co, :], thr_sb[:, 10 + co : 11 + co]
                )

            # full X^T in global key order: interleave the two pair blocks
            # (block p strip i of half h = global tile 2(4h+i)+p), 128-col
            # strips split across both HWDGE queues. The h=1 strips are
            # emitted later (before v_tiles(8..16)) so their semaphore waits
            # on the second gather don't clog the engine queues ahead of the
            # G^T / V-first-half compute.
            def x_strips(h, xg_h):
                xg_r = xg_h.rearrange("p (eo ei) b -> ei p eo b", ei=P)
                for p_ in range(2):
                    unpack_x(
                        xg_r[:, p_, :, 0:KSL].bitcast(i8),
                        xg_r[:, p_, :, KSL:GB],
                        [
                            xt[
                                :, :,
                                (2 * (4 * h + i) + p_) * P
                                : (2 * (4 * h + i) + p_ + 1) * P,
                            ]
                            for i in range(4)
                        ],
                        f"xt{h}{p_}",
                    )

            x_strips(0, xga)

            # --- projections ----------------------------------------------
            # G^T = wm^T Xq^T  (contract e over 8 co-chunks)
            for et in range(EO):
                for qh in range(2):
                    ps = psS.tile([P, KSL], f32, tag="ps", name="ps_gt")
                    for co in range(EO):
                        nc.tensor.matmul(
                            ps,
                            wm_sb[:, co, et * P : (et + 1) * P],
                            xq_sb[:, co, qh * KSL : (qh + 1) * KSL],
                            start=(co == 0),
                            stop=(co == EO - 1),
                        )
                    nc.scalar.copy(gt[:, et, qh * KSL : (qh + 1) * KSL], ps)

            def v_tiles(kt_range):
                # V = X Wv  (stationary X^T chunks, moving Wv)
                for kt in kt_range:
                    for dvh in range(2):
                        ps = psS.tile([P, KSL], f32, tag="ps", name="ps_v")
                        for eo in range(EO):
                            nc.tensor.matmul(
                                ps,
                                xt[:, eo, kt * P : (kt + 1) * P],
                                wv_sb[:, eo, dvh * KSL : (dvh + 1) * KSL],
                                start=(eo == 0),
                                stop=(eo == EO - 1),
                            )
                        nc.scalar.copy(v[:, kt, dvh * KSL : (dvh + 1) * KSL], ps)

            # --- attention over the 8 q-slots, interleaved with V halves
            # so slots 0-3 (k-tiles 0..7 only) run during the second X
            # half-gather. Slots 4-7's first two slabs also touch only
            # k-tiles 0..7, so they too are hoisted into phase 1 (their pt /
            # stats tiles persist across the phase boundary). ---------------
            with tc.tile_pool(name="attn", bufs=1) as attn:

                def slot_slabs(c):
                    # S in 512-wide slabs (256-slice pairs fused) plus a 256
                    # tail when c is odd; causal mask on the last 256 cols.
                    slabs = [(si * 2, 512) for si in range(c // 2)]
                    if c % 2:
                        slabs.append((c - 1, 256))
                    return slabs

                def attn_slot(s_slot, pt=None, stats=None, si_range=None):
                    c = CNT[s_slot]
                    if pt is None:
                        pt = attn.tile([P, 16, P], bf16, tag="pt", bufs=2)
                        stats = attn.tile([P, 14], f32, tag="stats", bufs=2)
                    slabs = slot_slabs(c)
                    nslab = len(slabs)
                    lo, hi = (0, nslab) if si_range is None else si_range
                    finish = hi == nslab
                    for si, (j0, width) in list(enumerate(slabs))[lo:hi]:
                        ps = psS.tile([P, KSL], f32, tag="ps", name="ps_s")[:, :width]
                        for eo in range(EO):
                            nc.tensor.matmul(
                                ps,
                                gt[:, eo, s_slot * P : (s_slot + 1) * P],
                                xt[:, eo, j0 * KA : j0 * KA + width],
                                start=(eo == 0),
                                stop=(eo == EO - 1),
                            )
                        if si == nslab - 1:
                            nc.vector.tensor_add(
                                ps[:, width - KA :], ps[:, width - KA :], masks_sb
                            )
                        p_sb = attn.tile([P, KSL], bf16, tag="p", bufs=3, name="p_sb")[
                            :, :width
                        ]
                        nc.scalar.activation(
                            p_sb,
                            ps,
                            mybir.ActivationFunctionType.Exp,
                            bias=0.0,
                            scale=1.0 / 32.0,
                            accum_out=stats[:, si : si + 1],
                        )
                        for t4 in range(width // P):
                            pst = psT.tile([P, P], bf16)
                            nc.tensor.transpose(
                                pst, p_sb[:, t4 * P : (t4 + 1) * P], ident
                            )
                            nc.vector.tensor_copy(pt[:, 2 * j0 + t4, :], pst)

                    if not finish:
                        return
                    nc.vector.reduce_sum(
                        stats[:, 8:9], stats[:, 0:nslab], axis=mybir.AxisListType.X
                    )
                    nc.vector.reciprocal(stats[:, 9:10], stats[:, 8:9])

                    o_f = attn.tile([P, D], f32, tag="of", bufs=2)
                    for dvh in range(2):
                        pso = psO.tile([P, KSL], f32, tag="o", name=f"pso_{dvh}")
                        for kt in range(2 * c):
                            nc.tensor.matmul(
                                pso,
                                pt[:, kt, :],
                                v[:, kt, dvh * KSL : (dvh + 1) * KSL],
                                start=(kt == 0),
                                stop=(kt == 2 * c - 1),
                            )
                        nc.vector.tensor_scalar_mul(
                            o_f[:, dvh * KSL : (dvh + 1) * KSL], pso, stats[:, 9:10]
                        )
                    # per-row |max| -> int7: u = round(o*63/rowmax)+64 in
                    # [1,127], then 8 u7 values bit-pack into 7 bytes
                    # (b_j = lowbits_{7-j}(u_j)*2^{j+1} + (u_{j+1} >> (6-j)),
                    # every intermediate <= 255 so no wraparound is needed)
                    nc.vector.reduce_max(
                        stats[:, 10:11],
                        o_f,
                        axis=mybir.AxisListType.X,
                        apply_absolute_value=True,
                    )
                    nc.vector.reciprocal(stats[:, 11:12], stats[:, 10:11])
                    nc.vector.tensor_scalar_mul(stats[:, 12:13], stats[:, 11:12], 63.0)
                    uq = attn.tile([P, D // 8, 8], u8, tag="uq", bufs=2)
                    nc.vector.tensor_scalar(
                        uq.rearrange("p g i -> p (g i)"),
                        o_f,
                        stats[:, 12:13],
                        64.0,
                        mybir.AluOpType.mult,
                        mybir.AluOpType.add,
                    )
                    o7 = attn.tile([P, D // 8, 7], u8, tag="o7", bufs=2)
                    shr = mybir.AluOpType.logical_shift_right
                    mul = mybir.AluOpType.mult
                    add = mybir.AluOpType.add
                    for j in range(7):
                        if j == 0:
                            mj = uq[:, :, 0]
                        else:
                            sj = attn.tile([P, D // 8], u8, tag="o7s", bufs=4)
                            mj = attn.tile([P, D // 8], u8, tag="o7m", bufs=4)
                            nc.vector.tensor_scalar(sj, uq[:, :, j], 7 - j, None, shr)
                            nc.vector.scalar_tensor_tensor(
                                mj, sj, -float(1 << (7 - j)), uq[:, :, j], mul, add
                            )
                        if j == 6:
                            tj = uq[:, :, 7]
                        else:
                            tj = attn.tile([P, D // 8], u8, tag="o7t", bufs=4)
                            nc.vector.tensor_scalar(tj, uq[:, :, j + 1], 6 - j, None, shr)
                        nc.vector.scalar_tensor_tensor(
                            o7[:, :, j], mj, float(1 << (j + 1)), tj, mul, add
                        )
                    scl_sb = attn.tile([P, 1], f32, tag="scl", bufs=2)
                    nc.vector.tensor_scalar_mul(scl_sb, stats[:, 10:11], 1.0 / 63.0)
                    nc.sync.dma_start(out_d[s_slot], o7.rearrange("p g i -> p (g i)"))
                    nc.scalar.dma_start(scl_d[s_slot], scl_sb)

                v_tiles(range(0, 8))
                for s in range(4):
                    attn_slot(s)
                # hoisted first-half slabs of slots 4-7 (k-tiles 0..7 only);
                # their pt/stats tiles persist into phase 2
                late = {}
                for s in range(4, 8):
                    late[s] = (
                        attn.tile([P, 16, P], bf16, tag=f"ptL{s}", name=f"ptL{s}"),
                        attn.tile([P, 14], f32, tag=f"stL{s}", name=f"stL{s}"),
                    )
                    attn_slot(s, *late[s], si_range=(0, 2))
                x_strips(1, xgb)
                v_tiles(range(8, 16))
                for s in range(4, 8):
                    attn_slot(s, *late[s], si_range=(2, len(slot_slabs(CNT[s]))))

    nc.compile()
    return nc


def _get_program():
    if "nc" not in _CACHE:
        _CACHE["nc"] = _build_program()
    return _CACHE["nc"]


def _get_exec():
    """Jitted shard_map executor with donation of the previous call's
    device-resident outputs (avoids uploading 8.4 MB of host zeros/call)."""
    if "exec" in _CACHE:
        return _CACHE["exec"]
    import jax
    from jax.experimental.shard_map import shard_map
    from jax.sharding import Mesh, PartitionSpec

    from concourse import mybir
    from concourse.bass2jax import (
        _bass_exec_p,
        install_neuronx_cc_hook,
        partition_id_tensor,
    )

    nc = _get_program()
    install_neuronx_cc_hook()

    partition_name = nc.partition_id_tensor.name if nc.partition_id_tensor else None
    in_names, out_names, out_avals = [], [], []
    for alloc in nc.m.functions[0].allocations:
        if not isinstance(alloc, mybir.MemoryLocationSet):
            continue
        name = alloc.memorylocations[0].name
        if alloc.kind == "ExternalInput":
            if name != partition_name:
                in_names.append(name)
        elif alloc.kind == "ExternalOutput":
            assert alloc.tensor_shape is not None and alloc.dtype is not None
            out_names.append(name)
            out_avals.append(
                jax.core.ShapedArray(tuple(alloc.tensor_shape), mybir.dt.np(alloc.dtype))
            )
    n_params = len(in_names)
    n_outs = len(out_names)
    in_names_all = in_names + out_names + ([partition_name] if partition_name else [])

    def _body(*args):
        operands = list(args)
        if partition_name is not None:
            operands.append(partition_id_tensor())
        outs = _bass_exec_p.bind(
            *operands,
            out_avals=tuple(out_avals),
            in_names=tuple(in_names_all),
            out_names=tuple(out_names),
            lowering_input_output_aliases=(),
            sim_require_finite=True,
            sim_require_nnan=True,
            nc=nc,
        )
        return tuple(outs)

    devices = jax.devices()[:NCORES]
    mesh = Mesh(np.asarray(devices), ("core",))
    donate = tuple(range(n_params, n_params + n_outs))
    sharded = jax.jit(
        shard_map(
            _body,
            mesh=mesh,
            in_specs=(PartitionSpec("core"),) * (n_params + n_outs),
            out_specs=(PartitionSpec("core"),) * n_outs,
            check_rep=False,
        ),
        donate_argnums=donate,
        keep_unused=True,
    )
    st = {
        "sharded": sharded,
        "in_names": in_names,
        "out_names": out_names,
        "out_avals": out_avals,
        "prev": None,
    }
    _CACHE["exec"] = st
    return st


def _run_once(maps):
    """One full call: upload inputs, execute on 8 cores, download outputs.

    Returns a per-core list of {name: np.ndarray} like run_bass_kernel_spmd.
    """
    st = _get_exec()
    in_names, out_names, out_avals = st["in_names"], st["out_names"], st["out_avals"]
    concat_in = [
        np.concatenate([np.asarray(m[n]) for m in maps], axis=0) for n in in_names
    ]
    don = st["prev"]
    if don is None:
        don = [
            np.zeros((NCORES * a.shape[0], *a.shape[1:]), a.dtype) for a in out_avals
        ]
    st["prev"] = None
    out_arrs = st["sharded"](*concat_in, *don)
    for o in out_arrs:
        o.copy_to_host_async()
    host = [np.asarray(o) for o in out_arrs]
    st["prev"] = list(out_arrs)
    return [
        {
            name: host[i].reshape(NCORES, *out_avals[i].shape)[c]
            for i, name in enumerate(out_names)
        }
        for c in range(NCORES)
    ]


def _pack_x10(xqt, s):
    """int10-pack [E, 1024] f32 (global power-of-2 scale s) into the
    [E, 1280] u8 wire layout: per 512-col group, 512 int8 high bytes
    (x10 >> 2) then 128 packed low bytes
    (byte j = lo[j] | lo[j+128]<<2 | lo[j+256]<<4 | lo[j+384]<<6)."""
    x10 = np.round(xqt / s).astype(np.int16)
    hi = (x10 >> 2).astype(np.int8)
    lo = (x10 & 3).astype(np.uint8)
    xp = np.empty((E, XW), np.uint8)
    for g in range(2):
        c0 = g * 512
        xp[:, g * GB : g * GB + KSL] = hi[:, c0 : c0 + 512].view(np.uint8)
        xp[:, g * GB + KSL : (g + 1) * GB] = (
            lo[:, c0 : c0 + 128]
            | (lo[:, c0 + 128 : c0 + 256] << 2)
            | (lo[:, c0 + 256 : c0 + 384] << 4)
            | (lo[:, c0 + 384 : c0 + 512] << 6)
        )
    return xp


def _in_maps(embeddings, Wq, Wk, Wv):
    wm = Wq.astype(np.float32) @ Wk.T.astype(np.float32)
    # per-row int8 quantization of wm; scales ride in thr cols 1..8
    s = np.abs(wm).max(axis=1) / 127.0  # [E]
    wm_i8 = np.clip(np.round(wm / s[:, None]), -127, 127).astype(np.int8)
    # per-row int8 quantization of Wv; scales in thr cols 10..17
    sv = np.abs(Wv).max(axis=1).astype(np.float32) / 127.0  # [E]
    wv_i8 = np.clip(np.round(Wv / sv[:, None]), -127, 127).astype(np.int8)
    shard = E // NCORES
    scl_cols = s.reshape(NCORES, P).T.astype(np.float32)  # [ci, co]
    sclv_cols = sv.reshape(NCORES, P).T.astype(np.float32)  # [ci, co]
    # global power-of-2 X scale: one bf16 rounding per dequantized element
    sx = float(2.0 ** np.ceil(np.log2(np.abs(embeddings).max() / 511.0)))
    maps = []
    for c in range(NCORES):
        b, g = divmod(c, 2)
        Xb = embeddings[b]
        xq = np.concatenate([Xb[P * t : P * (t + 1)] for t in TILES[g]], axis=0)
        thr = np.empty((P, 20), np.float32)
        thr[:, 0] = 128.0 * g
        thr[:, 1:9] = scl_cols
        thr[:, 9] = sx
        thr[:, 10:18] = sclv_cols
        thr[:, 18] = 4.0 * sx
        thr[:, 19] = -4.0 * sx
        maps.append(
            {
                "xp": _pack_x10(np.ascontiguousarray(xq.T).astype(np.float32), sx),
                "wmsh": np.ascontiguousarray(wm_i8[c * shard : (c + 1) * shard]),
                "wvsh": np.ascontiguousarray(wv_i8[c * shard : (c + 1) * shard]),
                "thr": thr,
            }
        )
    return maps


_U7_W = np.array([64, 32, 16, 8, 4, 2, 1], np.uint16)


def _gather_out(results):
    out = np.empty((B, S, D), np.float32)
    for c in range(NCORES):
        b, g = divmod(c, 2)
        o7 = np.asarray(results[c]["out"])  # [8, P, 896] u8
        scl = np.asarray(results[c]["scl"]).astype(np.float32)  # rowmax/63
        bits = np.unpackbits(o7.reshape(-1, 7), axis=1)  # [N, 56] MSB-first
        u = bits.reshape(-1, 8, 7).astype(np.uint16) @ _U7_W  # [N, 8]
        oc = u.reshape(8, P, D).astype(np.float32) - 64.0
        for s_slot, t in enumerate(TILES[g]):
            out[b, P * t : P * (t + 1), :] = oc[s_slot] * scl[s_slot]
    return out


def _run(embeddings, Wq, Wk, Wv):
    maps = _in_maps(embeddings, Wq, Wk, Wv)
    res = _run_once(maps)
    return _gather_out(res), res


def kernel(embeddings, Wq, Wk, Wv):
    embeddings = np.ascontiguousarray(np.asarray(embeddings, dtype=np.float32))
    Wq = np.ascontiguousarray(np.asarray(Wq, dtype=np.float32))
    Wk = np.ascontiguousarray(np.asarray(Wk, dtype=np.float32))
    Wv = np.ascontiguousarray(np.asarray(Wv, dtype=np.float32))
    out, _ = _run(embeddings, Wq, Wk, Wv)
    return out


# revision 11
# speedup vs baseline: 1.0587x; 1.0587x over previous
"""Causal attention kernel for Trainium2, SPMD over 8 NeuronCores.

Problem (hardcoded): embeddings [4, 2048, 1024] f32, Wq/Wk/Wv [1024, 1024] f32.
    q = X Wq; k = X Wk; v = X Wv
    out = softmax(causal(q k^T) / 32) v          (per batch)

Sharding: 8 cores = (4 batches) x (2 q-shards). Each core handles 1024 query
rows of one batch as eight 128-row q-tiles with balanced causal work:
core parity 0 gets the even global q-tiles [0,2,..,14], parity 1 the odd ones.
Both see the same per-slot k-extent pattern [1..8] (in 256-wide k-slices) and
a single causal-mask pattern (offset 0 or 128), so one SPMD program serves
all 8 cores; all per-core divergence is carried by input data.

Per-call host<->device traffic over the axon tunnel is the dominant cost
(~38 MB/s up, ~29 MB/s down, transfers serialized), so the kernel ships
every input byte exactly once at the smallest dtype that holds the 2e-2
error gate (host-simulated rel err for this config: 1.10e-2):
  - xp [1024, 1280] u8: the core's OWN q-tile columns of X^T (ascending tile
    order), int10-packed (per 512-col group: 512 int8 high bytes = x10>>2,
    then 128 bytes of 2-bit low fields, byte j = lo[j] | lo[j+128]<<2 |
    lo[j+256]<<4 | lo[j+384]<<6) with a global power-of-2 scale so the
    device unpack rounds only once in bf16. Used directly as Xq^T, AND
    pair-AllGathered in halves: the two rank blocks (even tiles | odd
    tiles) interleave back into the full X^T in global key order.
    1.25 MiB/core.
  - wmsh [128, 1024] int8 / wvsh [128, 1024] int8: the core's 1/8 row-shards
    of wm = Wq @ Wk.T and Wv (both per-row int8-quantized on host); each
    all-8 AllGathered on device and dequantized to bf16 in SBUF.
  - thr [128, 20] f32: col 0 parity threshold for the device-generated
    causal mask; cols 1..8 wm dequant scales; col 9 the X scale s;
    cols 10..17 wv dequant scales; col 18 = 4s; col 19 = -4s.
Output downloads as int7 (8 values bit-packed into 7 bytes, [8, 128, 896] u8)
with per-row f32 scales (0.875 MiB/core), dequantized on host. Host-simulated
rel err for the full config: 1.45e-2 vs the 2e-2 gate.

The runner bypasses run_bass_kernel_spmd: it jits the bass_exec shard_map
body directly so the output donation buffers can be the PREVIOUS call's
device-resident outputs (run_bass_kernel_spmd uploads 8.4 MB of host zeros
per call just to donate them), and fetches outputs with copy_to_host_async.

Device math (same algebraic structure as the fp32 reference):
    G^T = wm^T Xq^T; S = G X^T (slabwise, causal-masked); P = exp(S/32+mask)
    unnormalized with row-sums via activation accumulate; V = X Wv;
    O = (P V) * 1/rowsum.  All matmuls bf16 with fp32 PSUM accumulation.
"""

import numpy as np

B = 4
S = 2048
E = 1024
D = 1024
P = 128
NCORES = 8
KSL = 512  # k-slice width

XW = 1280  # int10-packed X row bytes (2 groups of 512+128)
GB = 640  # bytes per 512-col group

TILES = [
    [0, 2, 4, 6, 8, 10, 12, 14],
    [1, 3, 5, 7, 9, 11, 13, 15],
]
CNT = [1, 2, 3, 4, 5, 6, 7, 8]  # 256-wide k-slices per slot
KA = 256  # causal-mask tile width

MASK_VAL = -1.0e30

_CACHE = {}


def _build_program():
    import concourse.bacc as bacc
    import concourse.tile as tile
    from concourse import mybir
    from concourse.masks import make_identity

    bf16 = mybir.dt.bfloat16
    f32 = mybir.dt.float32

    nc = bacc.Bacc("TRN2", target_bir_lowering=False, debug=False, num_devices=NCORES)

    i8 = mybir.dt.int8
    u8 = mybir.dt.uint8
    # weights ship as int8 row-shards (per-row scales in thr); AllGathered.
    # Declared BEFORE xp so they upload first: the all-8 weight gathers and
    # the wm/wv dequant then overlap the 10.5 MB xp upload.
    wmsh_d = nc.dram_tensor("wmsh", [E // NCORES, D], i8, kind="ExternalInput")
    wvsh_d = nc.dram_tensor("wvsh", [E // NCORES, D], i8, kind="ExternalInput")
    # col 0: parity threshold for the causal mask; cols 1..8: wm per-row
    # dequant scales (row co*128+ci -> [ci, 1+co]); col 9: x scale s;
    # cols 10..17: wv scales; col 18: 4s; col 19: -4s
    thr_d = nc.dram_tensor("thr", [P, 20], f32, kind="ExternalInput")
    # X ships int10: per 512-col group, bytes [g*640, g*640+512) are the int8
    # high parts (x10 >> 2), bytes [g*640+512, (g+1)*640) the packed 2-bit
    # low fields (byte j = lo[j] | lo[j+128]<<2 | lo[j+256]<<4 | lo[j+384]<<6).
    # Global power-of-2 scale (thr col 9) -> unpack rounds once in bf16.
    xp_d = nc.dram_tensor("xp", [E, XW], u8, kind="ExternalInput")
    # int7 egress (8 values bit-packed into 7 bytes) with per-row dynamic
    # scale: 7/8 the download bytes of int8
    out_d = nc.dram_tensor("out", [8, P, (D // 8) * 7], u8, kind="ExternalOutput")
    scl_d = nc.dram_tensor("scl", [8, P, 1], f32, kind="ExternalOutput")

    EO = E // P  # 8 e-chunks
    KT = S // P  # 16 k-tiles
    PAIRS = [[0, 1], [2, 3], [4, 5], [6, 7]]
    ALL8 = [list(range(NCORES))]

    with tile.TileContext(nc) as tc:
        with (
            tc.tile_pool(name="dram", bufs=1, space="DRAM") as dram,
            tc.tile_pool(name="persist", bufs=1) as persist,
            tc.tile_pool(name="big", bufs=1) as big,
            tc.tile_pool(name="psS", bufs=3, space="PSUM") as psS,
            tc.tile_pool(name="psT", bufs=3, space="PSUM") as psT,
            tc.tile_pool(name="psO", bufs=2, space="PSUM") as psO,
            tc.tile_pool(name="xup", bufs=1) as xup,
        ):
            # --- bounce + collectives (weights first: G^T unblocks on it;
            # X pair-gather split in column halves so the first half's V +
            # attention slots 0-3 hide under the second half's gather) ------
            xq_bnc_a = dram.tile([E, GB], u8)
            xq_bnc_b = dram.tile([E, GB], u8)
            wm_bnc = dram.tile([E // NCORES, D], i8)
            wv_bnc = dram.tile([E // NCORES, D], i8)
            xga = dram.tile([2, E, GB], u8)
            xgb = dram.tile([2, E, GB], u8)
            wm8g = dram.tile([E, D], i8, addr_space="Shared")
            wvg = dram.tile([E, D], i8, addr_space="Shared")
            nc.gpsimd.dma_start(wm_bnc[:], wmsh_d[:])
            nc.gpsimd.dma_start(wv_bnc[:], wvsh_d[:])
            nc.gpsimd.dma_start(xq_bnc_a[:], xp_d[:, 0:GB])
            nc.gpsimd.dma_start(xq_bnc_b[:], xp_d[:, GB : 2 * GB])
            nc.gpsimd.collective_compute(
                "AllGather",
                mybir.AluOpType.bypass,
                replica_groups=ALL8,
                ins=[wm_bnc.opt()],
                outs=[wm8g.opt()],
            )
            nc.gpsimd.collective_compute(
                "AllGather",
                mybir.AluOpType.bypass,
                replica_groups=PAIRS,
                ins=[xq_bnc_a.opt()],
                outs=[xga.opt()],
            )
            nc.gpsimd.collective_compute(
                "AllGather",
                mybir.AluOpType.bypass,
                replica_groups=ALL8,
                ins=[wv_bnc.opt()],
                outs=[wvg.opt()],
            )
            nc.gpsimd.collective_compute(
                "AllGather",
                mybir.AluOpType.bypass,
                replica_groups=PAIRS,
                ins=[xq_bnc_b.opt()],
                outs=[xgb.opt()],
            )

            # --- SBUF residents -------------------------------------------
            ident = persist.tile([P, P], bf16, tag="ident")
            make_identity(nc, ident)
            # causal mask built on device: kill when col - row > 128*parity;
            # the parity rides in as a tiny [P, 1] threshold upload
            masks_sb = persist.tile([P, KA], f32, tag="masks")
            iota_sb = persist.tile([P, KA], f32, tag="iota")
            thr_sb = persist.tile([P, 20], f32, tag="thr")
            nc.sync.dma_start(thr_sb, thr_d[:])
            nc.gpsimd.iota(
                iota_sb,
                pattern=[[1, KA]],
                base=0,
                channel_multiplier=-1,
                allow_small_or_imprecise_dtypes=True,
            )
            nc.vector.tensor_scalar(
                masks_sb,
                iota_sb,
                thr_sb[:, 0:1],
                MASK_VAL,
                mybir.AluOpType.is_gt,
                mybir.AluOpType.mult,
            )

            xq_sb = persist.tile([P, EO, P * 8], bf16, tag="xq")  # Xq^T [e, q]
            wm_sb = persist.tile([P, EO, D], bf16, tag="wm")  # wm [e, e']
            wv_sb = persist.tile([P, EO, D], bf16, tag="wv")  # Wv [e, d]
            gt = persist.tile([P, EO, P * 8], bf16, tag="gt")  # G^T [e', q]
            xt = big.tile([P, EO, S], bf16, tag="xt")  # X^T [e, s]
            v = big.tile([P, KT, D], bf16, tag="v")  # V [k, d]

            # --- int10 X unpack: dst strip i gets hi_i*4s + lo_i*s where
            # lo_i = (b>>2i) - 4*(b>>2(i+2? no: 2(i+1))). Combine the lo
            # terms first (lo_i*s is exact in bf16, 2 bits) then add the
            # exact hi_i*4s, so the sum rounds only once — matching the
            # host-side error simulation.
            xscl = thr_sb[:, 9:10]  # s
            s4 = thr_sb[:, 18:19]  # 4s
            sm4 = thr_sb[:, 19:20]  # -4s

            def unpack_x(hi_src, lo_src, dst_strips, name):
                hi_sb = xup.tile([P, EO, KSL], i8, tag="uhi", name=f"uhi_{name}")
                lo_sb = xup.tile([P, EO, P], u8, tag="ulo", name=f"ulo_{name}")
                t_sh = [
                    xup.tile([P, EO, P], u8, tag=f"ut{k}", name=f"ut{k}_{name}")
                    for k in range(1, 4)
                ]
                tmp = xup.tile([P, EO, P], bf16, tag="utmp", name=f"utmp_{name}")
                u_sb = xup.tile([P, EO, P], bf16, tag="uu", name=f"uu_{name}")
                nc.sync.dma_start(hi_sb, hi_src)
                nc.scalar.dma_start(lo_sb, lo_src)
                ts = [lo_sb] + t_sh
                for k in range(1, 4):
                    nc.vector.tensor_scalar(
                        ts[k], lo_sb, 2 * k, None, mybir.AluOpType.logical_shift_right
                    )
                for i in range(4):
                    dst = dst_strips[i]
                    if i < 3:
                        nc.vector.tensor_scalar_mul(tmp, ts[i + 1], sm4)
                        nc.vector.scalar_tensor_tensor(
                            u_sb,
                            ts[i],
                            xscl,
                            tmp,
                            mybir.AluOpType.mult,
                            mybir.AluOpType.add,
                        )
                    else:
                        nc.vector.tensor_scalar_mul(u_sb, ts[3], xscl)
                    nc.vector.scalar_tensor_tensor(
                        dst,
                        hi_sb[:, :, i * P : (i + 1) * P],
                        s4,
                        u_sb,
                        mybir.AluOpType.mult,
                        mybir.AluOpType.add,
                    )

            # my own q columns: unpack straight from my upload
            xp_r = xp_d.rearrange("(eo ei) b -> ei eo b", ei=P)
            for h in range(2):
                unpack_x(
                    xp_r[:, :, h * GB : h * GB + KSL].bitcast(i8),
                    xp_r[:, :, h * GB + KSL : (h + 1) * GB],
                    [
                        xq_sb[:, :, h * KSL + i * P : h * KSL + (i + 1) * P]
                        for i in range(4)
                    ],
                    f"xq{h}",
                )

            # wm int8 from the first all-8 gather (rank block r = wm rows
            # r*128..(r+1)*128, i.e. co=r, ci=partition), dequantized to bf16
            # with the per-row scales from thr cols 1..8; wv int8 likewise
            # from the second gather with scales from cols 10..17
            wm8_r = wm8g.rearrange("(co ci) e -> ci co e", ci=P)
            wm_i8_sb = persist.tile([P, EO, D], i8, tag="wm8")
            nc.sync.dma_start(wm_i8_sb, wm8_r)
            for co in range(EO):
                nc.vector.tensor_scalar_mul(
                    wm_sb[:, co, :], wm_i8_sb[:, co, :], thr_sb[:, 1 + co : 2 + co]
                )
            wv_r = wvg.rearrange("(co ci) d -> ci co d", ci=P)
            wv_i8_sb = persist.tile([P, EO, D], i8, tag="wv8")
            nc.scalar.dma_start(wv_i8_sb, wv_r)
            for co in range(EO):
                nc.vector.tensor_scalar_mul(
                    wv_sb[:, co, :], wv_i8_sb[:, co, :], thr_sb[:, 10 + co : 11 + co]
                )

            # full X^T in global key order: interleave the two pair blocks
            # (block p strip i of half h = global tile 2(4h+i)+p), 128-col
            # strips split across both HWDGE queues. The h=1 strips are
            # emitted later (before v_tiles(8..16)) so their semaphore waits
            # on the second gather don't clog the engine queues ahead of the
            # G^T / V-first-half compute.
            def x_strips(h, xg_h):
                xg_r = xg_h.rearrange("p (eo ei) b -> ei p eo b", ei=P)
                for p_ in range(2):
                    unpack_x(
                        xg_r[:, p_, :, 0:KSL].bitcast(i8),
                        xg_r[:, p_, :, KSL:GB],
                        [
                            xt[
                                :, :,
                                (2 * (4 * h + i) + p_) * P
                                : (2 * (4 * h + i) + p_ + 1) * P,
                            ]
                            for i in range(4)
                        ],
                        f"xt{h}{p_}",
                    )

            x_strips(0, xga)

            # --- projections ----------------------------------------------
            # G^T = wm^T Xq^T  (contract e over 8 co-chunks)
            for et in range(EO):
                for qh in range(2):
                    ps = psS.tile([P, KSL], f32, tag="ps", name="ps_gt")
                    for co in range(EO):
                        nc.tensor.matmul(
                            ps,
                            wm_sb[:, co, et * P : (et + 1) * P],
                            xq_sb[:, co, qh * KSL : (qh + 1) * KSL],
                            start=(co == 0),
                            stop=(co == EO - 1),
                        )
                    nc.scalar.copy(gt[:, et, qh * KSL : (qh + 1) * KSL], ps)

            def v_tiles(kt_range):
                # V = X Wv  (stationary X^T chunks, moving Wv)
                for kt in kt_range:
                    for dvh in range(2):
                        ps = psS.tile([P, KSL], f32, tag="ps", name="ps_v")
                        for eo in range(EO):
                            nc.tensor.matmul(
                                ps,
                                xt[:, eo, kt * P : (kt + 1) * P],
                                wv_sb[:, eo, dvh * KSL : (dvh + 1) * KSL],
                                start=(eo == 0),
                                stop=(eo == EO - 1),
                            )
                        nc.scalar.copy(v[:, kt, dvh * KSL : (dvh + 1) * KSL], ps)

            # --- attention over the 8 q-slots, interleaved with V halves
            # so slots 0-3 (k-tiles 0..7 only) run during the second X
            # half-gather. Slots 4-7's first two slabs also touch only
            # k-tiles 0..7, so they too are hoisted into phase 1 (their pt /
            # stats tiles persist across the phase boundary). ---------------
            with tc.tile_pool(name="attn", bufs=1) as attn:

                def slot_slabs(c):
                    # S in 512-wide slabs (256-slice pairs fused) plus a 256
                    # tail when c is odd; causal mask on the last 256 cols.
                    slabs = [(si * 2, 512) for si in range(c // 2)]
                    if c % 2:
                        slabs.append((c - 1, 256))
                    return slabs

                def attn_slot(s_slot, pt=None, stats=None, si_range=None):
                    c = CNT[s_slot]
                    if pt is None:
                        pt = attn.tile([P, 16, P], bf16, tag="pt", bufs=2)
                        stats = attn.tile([P, 14], f32, tag="stats", bufs=2)
                    slabs = slot_slabs(c)
                    nslab = len(slabs)
                    lo, hi = (0, nslab) if si_range is None else si_range
                    finish = hi == nslab
                    for si, (j0, width) in list(enumerate(slabs))[lo:hi]:
                        ps = psS.tile([P, KSL], f32, tag="ps", name="ps_s")[:, :width]
                        for eo in range(EO):
                            nc.tensor.matmul(
                                ps,
                                gt[:, eo, s_slot * P : (s_slot + 1) * P],
                                xt[:, eo, j0 * KA : j0 * KA + width],
                                start=(eo == 0),
                                stop=(eo == EO - 1),
                            )
                        if si == nslab - 1:
                            nc.vector.tensor_add(
                                ps[:, width - KA :], ps[:, width - KA :], masks_sb
                            )
                        p_sb = attn.tile([P, KSL], bf16, tag="p", bufs=3, name="p_sb")[
                            :, :width
                        ]
                        nc.scalar.activation(
                            p_sb,
                            ps,
                            mybir.ActivationFunctionType.Exp,
                            bias=0.0,
                            scale=1.0 / 32.0,
                            accum_out=stats[:, si : si + 1],
                        )
                        for t4 in range(width // P):
                            pst = psT.tile([P, P], bf16)
                            nc.tensor.transpose(
                                pst, p_sb[:, t4 * P : (t4 + 1) * P], ident
                            )
                            nc.vector.tensor_copy(pt[:, 2 * j0 + t4, :], pst)

                    if not finish:
                        return
                    nc.vector.reduce_sum(
                        stats[:, 8:9], stats[:, 0:nslab], axis=mybir.AxisListType.X
                    )
                    nc.vector.reciprocal(stats[:, 9:10], stats[:, 8:9])

                    o_f = attn.tile([P, D], f32, tag="of", bufs=2)
                    for dvh in range(2):
                        pso = psO.tile([P, KSL], f32, tag="o", name=f"pso_{dvh}")
                        for kt in range(2 * c):
                            nc.tensor.matmul(
                                pso,
                                pt[:, kt, :],
                                v[:, kt, dvh * KSL : (dvh + 1) * KSL],
                                start=(kt == 0),
                                stop=(kt == 2 * c - 1),
                            )
                        nc.vector.tensor_scalar_mul(
                            o_f[:, dvh * KSL : (dvh + 1) * KSL], pso, stats[:, 9:10]
                        )
                    # per-row |max| -> int7: u = round(o*63/rowmax)+64 in
                    # [1,127], then 8 u7 values bit-pack into 7 bytes
                    # (b_j = lowbits_{7-j}(u_j)*2^{j+1} + (u_{j+1} >> (6-j)),
                    # every intermediate <= 255 so no wraparound is needed)
                    nc.vector.reduce_max(
                        stats[:, 10:11],
                        o_f,
                        axis=mybir.AxisListType.X,
                        apply_absolute_value=True,
                    )
                    nc.vector.reciprocal(stats[:, 11:12], stats[:, 10:11])
                    nc.vector.tensor_scalar_mul(stats[:, 12:13], stats[:, 11:12], 63.0)
                    uq = attn.tile([P, D // 8, 8], u8, tag="uq", bufs=2)
                    nc.vector.tensor_scalar(
                        uq.rearrange("p g i -> p (g i)"),
                        o_f,
                        stats[:, 12:13],
                        64.0,
                        mybir.AluOpType.mult,
                        mybir.AluOpType.add,
                    )
                    o7 = attn.tile([P, D // 8, 7], u8, tag="o7", bufs=2)
                    shr = mybir.AluOpType.logical_shift_right
                    mul = mybir.AluOpType.mult
                    add = mybir.AluOpType.add
                    for j in range(7):
                        if j == 0:
                            mj = uq[:, :, 0]
                        else:
                            sj = attn.tile([P, D // 8], u8, tag="o7s", bufs=4)
                            mj = attn.tile([P, D // 8], u8, tag="o7m", bufs=4)
                            nc.vector.tensor_scalar(sj, uq[:, :, j], 7 - j, None, shr)
                            nc.vector.scalar_tensor_tensor(
                                mj, sj, -float(1 << (7 - j)), uq[:, :, j], mul, add
                            )
                        if j == 6:
                            tj = uq[:, :, 7]
                        else:
                            tj = attn.tile([P, D // 8], u8, tag="o7t", bufs=4)
                            nc.vector.tensor_scalar(tj, uq[:, :, j + 1], 6 - j, None, shr)
                        nc.vector.scalar_tensor_tensor(
                            o7[:, :, j], mj, float(1 << (j + 1)), tj, mul, add
                        )
                    scl_sb = attn.tile([P, 1], f32, tag="scl", bufs=2)
                    nc.vector.tensor_scalar_mul(scl_sb, stats[:, 10:11], 1.0 / 63.0)
                    nc.sync.dma_start(out_d[s_slot], o7.rearrange("p g i -> p (g i)"))
                    nc.scalar.dma_start(scl_d[s_slot], scl_sb)

                v_tiles(range(0, 8))
                for s in range(4):
                    attn_slot(s)
                # hoisted first-half slabs of slots 4-7 (k-tiles 0..7 only);
                # their pt/stats tiles persist into phase 2
                late = {}
                for s in range(4, 8):
                    late[s] = (
                        attn.tile([P, 16, P], bf16, tag=f"ptL{s}", name=f"ptL{s}"),
                        attn.tile([P, 14], f32, tag=f"stL{s}", name=f"stL{s}"),
                    )
                    attn_slot(s, *late[s], si_range=(0, 2))
                x_strips(1, xgb)
                v_tiles(range(8, 16))
                for s in range(4, 8):
                    attn_slot(s, *late[s], si_range=(2, len(slot_slabs(CNT[s]))))

    nc.compile()
    return nc


def _get_program():
    if "nc" not in _CACHE:
        _CACHE["nc"] = _build_program()
    return _CACHE["nc"]


def _get_exec():
    """Jitted shard_map executor with donation of the previous call's
    device-resident outputs (avoids uploading 8.4 MB of host zeros/call)."""
    if "exec" in _CACHE:
        return _CACHE["exec"]
    import jax
    from jax.experimental.shard_map import shard_map
    from jax.sharding import Mesh, PartitionSpec

    from concourse import mybir
    from concourse.bass2jax import (
        _bass_exec_p,
        install_neuronx_cc_hook,
        partition_id_tensor,
    )

    nc = _get_program()
    install_neuronx_cc_hook()

    partition_name = nc.partition_id_tensor.name if nc.partition_id_tensor else None
    in_names, out_names, out_avals = [], [], []
    for alloc in nc.m.functions[0].allocations:
        if not isinstance(alloc, mybir.MemoryLocationSet):
            continue
        name = alloc.memorylocations[0].name
        if alloc.kind == "ExternalInput":
            if name != partition_name:
                in_names.append(name)
        elif alloc.kind == "ExternalOutput":
            assert alloc.tensor_shape is not None and alloc.dtype is not None
            out_names.append(name)
            out_avals.append(
                jax.core.ShapedArray(tuple(alloc.tensor_shape), mybir.dt.np(alloc.dtype))
            )
    n_params = len(in_names)
    n_outs = len(out_names)
    in_names_all = in_names + out_names + ([partition_name] if partition_name else [])

    def _body(*args):
        operands = list(args)
        if partition_name is not None:
            operands.append(partition_id_tensor())
        outs = _bass_exec_p.bind(
            *operands,
            out_avals=tuple(out_avals),
            in_names=tuple(in_names_all),
            out_names=tuple(out_names),
            lowering_input_output_aliases=(),
            sim_require_finite=True,
            sim_require_nnan=True,
            nc=nc,
        )
        return tuple(outs)

    devices = jax.devices()[:NCORES]
    mesh = Mesh(np.asarray(devices), ("core",))
    donate = tuple(range(n_params, n_params + n_outs))
    sharded = jax.jit(
        shard_map(
            _body,
            mesh=mesh,
            in_specs=(PartitionSpec("core"),) * (n_params + n_outs),
            out_specs=(PartitionSpec("core"),) * n_outs,
            check_rep=False,
        ),
        donate_argnums=donate,
        keep_unused=True,
    )
    st = {
        "sharded": sharded,
        "in_names": in_names,
        "out_names": out_names,
        "out_avals": out_avals,
        "prev": None,
    }
    _CACHE["exec"] = st
    return st


def _run_once(maps):
    """One full call: upload inputs, execute on 8 cores, download outputs.

    Returns a per-core list of {name: np.ndarray} like run_bass_kernel_spmd.
    """
    st = _get_exec()
    in_names, out_names, out_avals = st["in_names"], st["out_names"], st["out_avals"]
    concat_in = [
        np.concatenate([np.asarray(m[n]) for m in maps], axis=0) for n in in_names
    ]
    don = st["prev"]
    if don is None:
        don = [
            np.zeros((NCORES * a.shape[0], *a.shape[1:]), a.dtype) for a in out_avals
        ]
    st["prev"] = None
    out_arrs = st["sharded"](*concat_in, *don)
    for o in out_arrs:
        o.copy_to_host_async()
    host = [np.asarray(o) for o in out_arrs]
    st["prev"] = list(out_arrs)
    return [
        {
            name: host[i].reshape(NCORES, *out_avals[i].shape)[c]
            for i, name in enumerate(out_names)
        }
        for c in range(NCORES)
    ]


def _pack_x10(xqt, s):
    """int10-pack [E, 1024] f32 (global power-of-2 scale s) into the
    [E, 1280] u8 wire layout: per 512-col group, 512 int8 high bytes
    (x10 >> 2) then 128 packed low bytes
    (byte j = lo[j] | lo[j+128]<<2 | lo[j+256]<<4 | lo[j+384]<<6)."""
    x10 = np.round(xqt / s).astype(np.int16)
    hi = (x10 >> 2).astype(np.int8)
    lo = (x10 & 3).astype(np.uint8)
    xp = np.empty((E, XW), np.uint8)
    for g in range(2):
        c0 = g * 512
        xp[:, g * GB : g * GB + KSL] = hi[:, c0 : c0 + 512].view(np.uint8)
        xp[:, g * GB + KSL : (g + 1) * GB] = (
            lo[:, c0 : c0 + 128]
            | (lo[:, c0 + 128 : c0 + 256] << 2)
            | (lo[:, c0 + 256 : c0 + 384] << 4)
            | (lo[:, c0 + 384 : c0 + 512] << 6)
        )
    return xp


def _in_maps(embeddings, Wq, Wk, Wv):
    wm = Wq.astype(np.float32) @ Wk.T.astype(np.float32)
    # per-row int8 quantization of wm; scales ride in thr cols 1..8
    s = np.abs(wm).max(axis=1) / 127.0  # [E]
    wm_i8 = np.clip(np.round(wm / s[:, None]), -127, 127).astype(np.int8)
    # per-row int8 quantization of Wv; scales in thr cols 10..17
    sv = np.abs(Wv).max(axis=1).astype(np.float32) / 127.0  # [E]
    wv_i8 = np.clip(np.round(Wv / sv[:, None]), -127, 127).astype(np.int8)
    shard = E // NCORES
    scl_cols = s.reshape(NCORES, P).T.astype(np.float32)  # [ci, co]
    sclv_cols = sv.reshape(NCORES, P).T.astype(np.float32)  # [ci, co]
    # global power-of-2 X scale: one bf16 rounding per dequantized element
    sx = float(2.0 ** np.ceil(np.log2(np.abs(embeddings).max() / 511.0)))
    maps = []
    for c in range(NCORES):
        b, g = divmod(c, 2)
        Xb = embeddings[b]
        xq = np.concatenate([Xb[P * t : P * (t + 1)] for t in TILES[g]], axis=0)
        thr = np.empty((P, 20), np.float32)
        thr[:, 0] = 128.0 * g
        thr[:, 1:9] = scl_cols
        thr[:, 9] = sx
        thr[:, 10:18] = sclv_cols
        thr[:, 18] = 4.0 * sx
        thr[:, 19] = -4.0 * sx
        maps.append(
            {
                "xp": _pack_x10(np.ascontiguousarray(xq.T).astype(np.float32), sx),
                "wmsh": np.ascontiguousarray(wm_i8[c * shard : (c + 1) * shard]),
                "wvsh": np.ascontiguousarray(wv_i8[c * shard : (c + 1) * shard]),
                "thr": thr,
            }
        )
    return maps


_U7_W = np.array([64, 32, 16, 8, 4, 2, 1], np.uint16)


def _gather_out(results):
    out = np.empty((B, S, D), np.float32)
    for c in range(NCORES):
        b, g = divmod(c, 2)
        o7 = np.asarray(results[c]["out"])  # [8, P, 896] u8
        scl = np.asarray(results[c]["scl"]).astype(np.float32)  # rowmax/63
        bits = np.unpackbits(o7.reshape(-1, 7), axis=1)  # [N, 56] MSB-first
        u = bits.reshape(-1, 8, 7).astype(np.uint16) @ _U7_W  # [N, 8]
        oc = u.reshape(8, P, D).astype(np.float32) - 64.0
        for s_slot, t in enumerate(TILES[g]):
            out[b, P * t : P * (t + 1), :] = oc[s_slot] * scl[s_slot]
    return out


def _run(embeddings, Wq, Wk, Wv):
    maps = _in_maps(embeddings, Wq, Wk, Wv)
    res = _run_once(maps)
    return _gather_out(res), res


def kernel(embeddings, Wq, Wk, Wv):
    embeddings = np.ascontiguousarray(np.asarray(embeddings, dtype=np.float32))
    Wq = np.ascontiguousarray(np.asarray(Wq, dtype=np.float32))
    Wk = np.ascontiguousarray(np.asarray(Wk, dtype=np.float32))
    Wv = np.ascontiguousarray(np.asarray(Wv, dtype=np.float32))
    out, _ = _run(embeddings, Wq, Wk, Wv)
    return out


# revision 14
# speedup vs baseline: 1.0603x; 1.0016x over previous
"""Causal attention kernel for Trainium2, SPMD over 8 NeuronCores.

Problem (hardcoded): embeddings [4, 2048, 1024] f32, Wq/Wk/Wv [1024, 1024] f32.
    q = X Wq; k = X Wk; v = X Wv
    out = softmax(causal(q k^T) / 32) v          (per batch)

Sharding: 8 cores = (4 batches) x (2 q-shards). Each core handles 1024 query
rows of one batch as eight 128-row q-tiles with balanced causal work:
core parity 0 gets the even global q-tiles [0,2,..,14], parity 1 the odd ones.
Both see the same per-slot k-extent pattern [1..8] (in 256-wide k-slices) and
a single causal-mask pattern (offset 0 or 128), so one SPMD program serves
all 8 cores; all per-core divergence is carried by input data.

Per-call host<->device traffic over the axon tunnel is the dominant cost
(~38 MB/s up, ~29 MB/s down, transfers serialized), so the kernel ships
every input byte exactly once at the smallest dtype that holds the 2e-2
error gate (host-simulated rel err for this config: 1.10e-2):
  - xp [1024, 1280] u8: the core's OWN q-tile columns of X^T (ascending tile
    order), int10-packed (per 512-col group: 512 int8 high bytes = x10>>2,
    then 128 bytes of 2-bit low fields, byte j = lo[j] | lo[j+128]<<2 |
    lo[j+256]<<4 | lo[j+384]<<6) with a global power-of-2 scale so the
    device unpack rounds only once in bf16. Used directly as Xq^T, AND
    pair-AllGathered in halves: the two rank blocks (even tiles | odd
    tiles) interleave back into the full X^T in global key order.
    1.25 MiB/core.
  - wmsh [128, 1024] int8 / wvsh [128, 1024] int8: the core's 1/8 row-shards
    of wm = Wq @ Wk.T and Wv (both per-row int8-quantized on host); each
    all-8 AllGathered on device and dequantized to bf16 in SBUF.
  - thr [128, 20] f32: col 0 parity threshold for the device-generated
    causal mask; cols 1..8 wm dequant scales; col 9 the X scale s;
    cols 10..17 wv dequant scales; col 18 = 4s; col 19 = -4s.
Output downloads as int7 (8 values bit-packed into 7 bytes, [8, 128, 896] u8)
with per-row f32 scales (0.875 MiB/core), dequantized on host. Host-simulated
rel err for the full config: 1.45e-2 vs the 2e-2 gate.

The runner bypasses run_bass_kernel_spmd: it jits the bass_exec shard_map
body directly so the output donation buffers can be the PREVIOUS call's
device-resident outputs (run_bass_kernel_spmd uploads 8.4 MB of host zeros
per call just to donate them), and fetches outputs with copy_to_host_async.

Device math (same algebraic structure as the fp32 reference):
    G^T = wm^T Xq^T; S = G X^T (slabwise, causal-masked); P = exp(S/32+mask)
    unnormalized with row-sums via activation accumulate; V = X Wv;
    O = (P V) * 1/rowsum.  All matmuls bf16 with fp32 PSUM accumulation.
"""

import numpy as np

B = 4
S = 2048
E = 1024
D = 1024
P = 128
NCORES = 8
KSL = 512  # k-slice width

XW = 1280  # int10-packed X row bytes (2 groups of 512+128)
GB = 640  # bytes per 512-col group

TILES = [
    [0, 2, 4, 6, 8, 10, 12, 14],
    [1, 3, 5, 7, 9, 11, 13, 15],
]
CNT = [1, 2, 3, 4, 5, 6, 7, 8]  # 256-wide k-slices per slot
KA = 256  # causal-mask tile width

MASK_VAL = -1.0e30

_CACHE = {}


def _build_program():
    import concourse.bacc as bacc
    import concourse.tile as tile
    from concourse import mybir
    from concourse.masks import make_identity

    bf16 = mybir.dt.bfloat16
    f32 = mybir.dt.float32

    nc = bacc.Bacc("TRN2", target_bir_lowering=False, debug=False, num_devices=NCORES)

    i8 = mybir.dt.int8
    u8 = mybir.dt.uint8
    # weights ship as int8 row-shards (per-row scales in thr); AllGathered.
    # Declared BEFORE xp so they upload first: the all-8 weight gathers and
    # the wm/wv dequant then overlap the 10.5 MB xp upload.
    wmsh_d = nc.dram_tensor("wmsh", [E // NCORES, D], i8, kind="ExternalInput")
    wvsh_d = nc.dram_tensor("wvsh", [E // NCORES, D], i8, kind="ExternalInput")
    # col 0: parity threshold for the causal mask; cols 1..8: wm per-row
    # dequant scales (row co*128+ci -> [ci, 1+co]); col 9: x scale s;
    # cols 10..17: wv scales; col 18: 4s; col 19: -4s
    thr_d = nc.dram_tensor("thr", [P, 20], f32, kind="ExternalInput")
    # X ships int10: per 512-col group, bytes [g*640, g*640+512) are the int8
    # high parts (x10 >> 2), bytes [g*640+512, (g+1)*640) the packed 2-bit
    # low fields (byte j = lo[j] | lo[j+128]<<2 | lo[j+256]<<4 | lo[j+384]<<6).
    # Global power-of-2 scale (thr col 9) -> unpack rounds once in bf16.
    xp_d = nc.dram_tensor("xp", [E, XW], u8, kind="ExternalInput")
    # int7 egress (8 values bit-packed into 7 bytes) with per-row dynamic
    # scale: 7/8 the download bytes of int8. The f32 scale rides in the
    # last 4 bytes of each row, so there is a single output array to fetch.
    OW = (D // 8) * 7 + 4
    out_d = nc.dram_tensor("out", [8, P, OW], u8, kind="ExternalOutput")

    EO = E // P  # 8 e-chunks
    KT = S // P  # 16 k-tiles
    PAIRS = [[0, 1], [2, 3], [4, 5], [6, 7]]
    ALL8 = [list(range(NCORES))]

    with tile.TileContext(nc) as tc:
        with (
            tc.tile_pool(name="dram", bufs=1, space="DRAM") as dram,
            tc.tile_pool(name="persist", bufs=1) as persist,
            tc.tile_pool(name="big", bufs=1) as big,
            tc.tile_pool(name="psS", bufs=3, space="PSUM") as psS,
            tc.tile_pool(name="psT", bufs=3, space="PSUM") as psT,
            tc.tile_pool(name="psO", bufs=2, space="PSUM") as psO,
            tc.tile_pool(name="xup", bufs=1) as xup,
        ):
            # --- bounce + collectives (weights first: G^T unblocks on it;
            # X pair-gather split in column halves so the first half's V +
            # attention slots 0-3 hide under the second half's gather) ------
            xq_bnc_a = dram.tile([E, GB], u8)
            xq_bnc_b = dram.tile([E, GB], u8)
            wm_bnc = dram.tile([E // NCORES, D], i8)
            wv_bnc = dram.tile([E // NCORES, D], i8)
            xga = dram.tile([2, E, GB], u8)
            xgb = dram.tile([2, E, GB], u8)
            wm8g = dram.tile([E, D], i8, addr_space="Shared")
            wvg = dram.tile([E, D], i8, addr_space="Shared")
            nc.gpsimd.dma_start(wm_bnc[:], wmsh_d[:])
            nc.gpsimd.dma_start(wv_bnc[:], wvsh_d[:])
            nc.gpsimd.dma_start(xq_bnc_a[:], xp_d[:, 0:GB])
            nc.gpsimd.dma_start(xq_bnc_b[:], xp_d[:, GB : 2 * GB])
            nc.gpsimd.collective_compute(
                "AllGather",
                mybir.AluOpType.bypass,
                replica_groups=ALL8,
                ins=[wm_bnc.opt()],
                outs=[wm8g.opt()],
            )
            nc.gpsimd.collective_compute(
                "AllGather",
                mybir.AluOpType.bypass,
                replica_groups=PAIRS,
                ins=[xq_bnc_a.opt()],
                outs=[xga.opt()],
            )
            nc.gpsimd.collective_compute(
                "AllGather",
                mybir.AluOpType.bypass,
                replica_groups=ALL8,
                ins=[wv_bnc.opt()],
                outs=[wvg.opt()],
            )
            nc.gpsimd.collective_compute(
                "AllGather",
                mybir.AluOpType.bypass,
                replica_groups=PAIRS,
                ins=[xq_bnc_b.opt()],
                outs=[xgb.opt()],
            )

            # --- SBUF residents -------------------------------------------
            ident = persist.tile([P, P], bf16, tag="ident")
            make_identity(nc, ident)
            # causal mask built on device: kill when col - row > 128*parity;
            # the parity rides in as a tiny [P, 1] threshold upload
            masks_sb = persist.tile([P, KA], f32, tag="masks")
            iota_sb = persist.tile([P, KA], f32, tag="iota")
            thr_sb = persist.tile([P, 20], f32, tag="thr")
            nc.sync.dma_start(thr_sb, thr_d[:])
            nc.gpsimd.iota(
                iota_sb,
                pattern=[[1, KA]],
                base=0,
                channel_multiplier=-1,
                allow_small_or_imprecise_dtypes=True,
            )
            nc.vector.tensor_scalar(
                masks_sb,
                iota_sb,
                thr_sb[:, 0:1],
                MASK_VAL,
                mybir.AluOpType.is_gt,
                mybir.AluOpType.mult,
            )

            xq_sb = persist.tile([P, EO, P * 8], bf16, tag="xq")  # Xq^T [e, q]
            wm_sb = persist.tile([P, EO, D], bf16, tag="wm")  # wm [e, e']
            wv_sb = persist.tile([P, EO, D], bf16, tag="wv")  # Wv [e, d]
            gt = persist.tile([P, EO, P * 8], bf16, tag="gt")  # G^T [e', q]
            xt = big.tile([P, EO, S], bf16, tag="xt")  # X^T [e, s]
            v = big.tile([P, KT, D], bf16, tag="v")  # V [k, d]

            # --- int10 X unpack: dst strip i gets hi_i*4s + lo_i*s where
            # lo_i = (b>>2i) - 4*(b>>2(i+2? no: 2(i+1))). Combine the lo
            # terms first (lo_i*s is exact in bf16, 2 bits) then add the
            # exact hi_i*4s, so the sum rounds only once — matching the
            # host-side error simulation.
            xscl = thr_sb[:, 9:10]  # s
            s4 = thr_sb[:, 18:19]  # 4s
            sm4 = thr_sb[:, 19:20]  # -4s

            def unpack_x(hi_src, lo_src, dst_strips, name):
                hi_sb = xup.tile([P, EO, KSL], i8, tag="uhi", name=f"uhi_{name}")
                lo_sb = xup.tile([P, EO, P], u8, tag="ulo", name=f"ulo_{name}")
                t_sh = [
                    xup.tile([P, EO, P], u8, tag=f"ut{k}", name=f"ut{k}_{name}")
                    for k in range(1, 4)
                ]
                tmp = xup.tile([P, EO, P], bf16, tag="utmp", name=f"utmp_{name}")
                u_sb = xup.tile([P, EO, P], bf16, tag="uu", name=f"uu_{name}")
                nc.sync.dma_start(hi_sb, hi_src)
                nc.scalar.dma_start(lo_sb, lo_src)
                ts = [lo_sb] + t_sh
                for k in range(1, 4):
                    nc.vector.tensor_scalar(
                        ts[k], lo_sb, 2 * k, None, mybir.AluOpType.logical_shift_right
                    )
                for i in range(4):
                    dst = dst_strips[i]
                    if i < 3:
                        nc.vector.tensor_scalar_mul(tmp, ts[i + 1], sm4)
                        nc.vector.scalar_tensor_tensor(
                            u_sb,
                            ts[i],
                            xscl,
                            tmp,
                            mybir.AluOpType.mult,
                            mybir.AluOpType.add,
                        )
                    else:
                        nc.vector.tensor_scalar_mul(u_sb, ts[3], xscl)
                    nc.vector.scalar_tensor_tensor(
                        dst,
                        hi_sb[:, :, i * P : (i + 1) * P],
                        s4,
                        u_sb,
                        mybir.AluOpType.mult,
                        mybir.AluOpType.add,
                    )

            # my own q columns: unpack straight from my upload
            xp_r = xp_d.rearrange("(eo ei) b -> ei eo b", ei=P)
            for h in range(2):
                unpack_x(
                    xp_r[:, :, h * GB : h * GB + KSL].bitcast(i8),
                    xp_r[:, :, h * GB + KSL : (h + 1) * GB],
                    [
                        xq_sb[:, :, h * KSL + i * P : h * KSL + (i + 1) * P]
                        for i in range(4)
                    ],
                    f"xq{h}",
                )

            # wm int8 from the first all-8 gather (rank block r = wm rows
            # r*128..(r+1)*128, i.e. co=r, ci=partition), dequantized to bf16
            # with the per-row scales from thr cols 1..8; wv int8 likewise
            # from the second gather with scales from cols 10..17
            wm8_r = wm8g.rearrange("(co ci) e -> ci co e", ci=P)
            wm_i8_sb = persist.tile([P, EO, D], i8, tag="wm8")
            nc.sync.dma_start(wm_i8_sb, wm8_r)
            for co in range(EO):
                nc.vector.tensor_scalar_mul(
                    wm_sb[:, co, :], wm_i8_sb[:, co, :], thr_sb[:, 1 + co : 2 + co]
                )
            wv_r = wvg.rearrange("(co ci) d -> ci co d", ci=P)
            wv_i8_sb = persist.tile([P, EO, D], i8, tag="wv8")
            nc.scalar.dma_start(wv_i8_sb, wv_r)
            for co in range(EO):
                nc.vector.tensor_scalar_mul(
                    wv_sb[:, co, :], wv_i8_sb[:, co, :], thr_sb[:, 10 + co : 11 + co]
                )

            # full X^T in global key order: interleave the two pair blocks
            # (block p strip i of half h = global tile 2(4h+i)+p), 128-col
            # strips split across both HWDGE queues. The h=1 strips are
            # emitted later (before v_tiles(8..16)) so their semaphore waits
            # on the second gather don't clog the engine queues ahead of the
            # G^T / V-first-half compute.
            def x_strips(h, xg_h):
                xg_r = xg_h.rearrange("p (eo ei) b -> ei p eo b", ei=P)
                for p_ in range(2):
                    unpack_x(
                        xg_r[:, p_, :, 0:KSL].bitcast(i8),
                        xg_r[:, p_, :, KSL:GB],
                        [
                            xt[
                                :, :,
                                (2 * (4 * h + i) + p_) * P
                                : (2 * (4 * h + i) + p_ + 1) * P,
                            ]
                            for i in range(4)
                        ],
                        f"xt{h}{p_}",
                    )

            x_strips(0, xga)

            # --- projections ----------------------------------------------
            # G^T = wm^T Xq^T  (contract e over 8 co-chunks)
            for et in range(EO):
                for qh in range(2):
                    ps = psS.tile([P, KSL], f32, tag="ps", name="ps_gt")
                    for co in range(EO):
                        nc.tensor.matmul(
                            ps,
                            wm_sb[:, co, et * P : (et + 1) * P],
                            xq_sb[:, co, qh * KSL : (qh + 1) * KSL],
                            start=(co == 0),
                            stop=(co == EO - 1),
                        )
                    nc.scalar.copy(gt[:, et, qh * KSL : (qh + 1) * KSL], ps)

            def v_tiles(kt_range):
                # V = X Wv  (stationary X^T chunks, moving Wv)
                for kt in kt_range:
                    for dvh in range(2):
                        ps = psS.tile([P, KSL], f32, tag="ps", name="ps_v")
                        for eo in range(EO):
                            nc.tensor.matmul(
                                ps,
                                xt[:, eo, kt * P : (kt + 1) * P],
                                wv_sb[:, eo, dvh * KSL : (dvh + 1) * KSL],
                                start=(eo == 0),
                                stop=(eo == EO - 1),
                            )
                        nc.scalar.copy(v[:, kt, dvh * KSL : (dvh + 1) * KSL], ps)

            # --- attention over the 8 q-slots, interleaved with V halves
            # so slots 0-3 (k-tiles 0..7 only) run during the second X
            # half-gather. Slots 4-7's first two slabs also touch only
            # k-tiles 0..7, so they too are hoisted into phase 1 (their pt /
            # stats tiles persist across the phase boundary). ---------------
            with tc.tile_pool(name="attn", bufs=1) as attn:

                def slot_slabs(c):
                    # S in 512-wide slabs (256-slice pairs fused) plus a 256
                    # tail when c is odd; causal mask on the last 256 cols.
                    slabs = [(si * 2, 512) for si in range(c // 2)]
                    if c % 2:
                        slabs.append((c - 1, 256))
                    return slabs

                def attn_slot(s_slot, pt=None, stats=None, si_range=None):
                    c = CNT[s_slot]
                    if pt is None:
                        pt = attn.tile([P, 16, P], bf16, tag="pt", bufs=2)
                        stats = attn.tile([P, 14], f32, tag="stats", bufs=2)
                    slabs = slot_slabs(c)
                    nslab = len(slabs)
                    lo, hi = (0, nslab) if si_range is None else si_range
                    finish = hi == nslab
                    for si, (j0, width) in list(enumerate(slabs))[lo:hi]:
                        ps = psS.tile([P, KSL], f32, tag="ps", name="ps_s")[:, :width]
                        for eo in range(EO):
                            nc.tensor.matmul(
                                ps,
                                gt[:, eo, s_slot * P : (s_slot + 1) * P],
                                xt[:, eo, j0 * KA : j0 * KA + width],
                                start=(eo == 0),
                                stop=(eo == EO - 1),
                            )
                        if si == nslab - 1:
                            nc.vector.tensor_add(
                                ps[:, width - KA :], ps[:, width - KA :], masks_sb
                            )
                        p_sb = attn.tile([P, KSL], bf16, tag="p", bufs=3, name="p_sb")[
                            :, :width
                        ]
                        nc.scalar.activation(
                            p_sb,
                            ps,
                            mybir.ActivationFunctionType.Exp,
                            bias=0.0,
                            scale=1.0 / 32.0,
                            accum_out=stats[:, si : si + 1],
                        )
                        for t4 in range(width // P):
                            pst = psT.tile([P, P], bf16)
                            nc.tensor.transpose(
                                pst, p_sb[:, t4 * P : (t4 + 1) * P], ident
                            )
                            nc.vector.tensor_copy(pt[:, 2 * j0 + t4, :], pst)

                    if not finish:
                        return
                    nc.vector.reduce_sum(
                        stats[:, 8:9], stats[:, 0:nslab], axis=mybir.AxisListType.X
                    )
                    nc.vector.reciprocal(stats[:, 9:10], stats[:, 8:9])

                    o_f = attn.tile([P, D], f32, tag="of", bufs=2)
                    for dvh in range(2):
                        pso = psO.tile([P, KSL], f32, tag="o", name=f"pso_{dvh}")
                        for kt in range(2 * c):
                            nc.tensor.matmul(
                                pso,
                                pt[:, kt, :],
                                v[:, kt, dvh * KSL : (dvh + 1) * KSL],
                                start=(kt == 0),
                                stop=(kt == 2 * c - 1),
                            )
                        nc.vector.tensor_scalar_mul(
                            o_f[:, dvh * KSL : (dvh + 1) * KSL], pso, stats[:, 9:10]
                        )
                    # per-row |max| -> int7: u = round(o*63/rowmax)+64 in
                    # [1,127], then 8 u7 values bit-pack into 7 bytes
                    # (b_j = lowbits_{7-j}(u_j)*2^{j+1} + (u_{j+1} >> (6-j)),
                    # every intermediate <= 255 so no wraparound is needed)
                    nc.vector.reduce_max(
                        stats[:, 10:11],
                        o_f,
                        axis=mybir.AxisListType.X,
                        apply_absolute_value=True,
                    )
                    nc.vector.reciprocal(stats[:, 11:12], stats[:, 10:11])
                    nc.vector.tensor_scalar_mul(stats[:, 12:13], stats[:, 11:12], 63.0)
                    uq = attn.tile([P, D // 8, 8], u8, tag="uq", bufs=2)
                    nc.vector.tensor_scalar(
                        uq.rearrange("p g i -> p (g i)"),
                        o_f,
                        stats[:, 12:13],
                        64.0,
                        mybir.AluOpType.mult,
                        mybir.AluOpType.add,
                    )
                    o7 = attn.tile([P, D // 8, 7], u8, tag="o7", bufs=2)
                    shr = mybir.AluOpType.logical_shift_right
                    mul = mybir.AluOpType.mult
                    add = mybir.AluOpType.add
                    for j in range(7):
                        if j == 0:
                            mj = uq[:, :, 0]
                        else:
                            sj = attn.tile([P, D // 8], u8, tag="o7s", bufs=4)
                            mj = attn.tile([P, D // 8], u8, tag="o7m", bufs=4)
                            nc.vector.tensor_scalar(sj, uq[:, :, j], 7 - j, None, shr)
                            nc.vector.scalar_tensor_tensor(
                                mj, sj, -float(1 << (7 - j)), uq[:, :, j], mul, add
                            )
                        if j == 6:
                            tj = uq[:, :, 7]
                        else:
                            tj = attn.tile([P, D // 8], u8, tag="o7t", bufs=4)
                            nc.vector.tensor_scalar(tj, uq[:, :, j + 1], 6 - j, None, shr)
                        nc.vector.scalar_tensor_tensor(
                            o7[:, :, j], mj, float(1 << (j + 1)), tj, mul, add
                        )
                    scl_sb = attn.tile([P, 1], f32, tag="scl", bufs=2)
                    nc.vector.tensor_scalar_mul(scl_sb, stats[:, 10:11], 1.0 / 63.0)
                    nc.sync.dma_start(
                        out_d[s_slot, :, 0 : OW - 4],
                        o7.rearrange("p g i -> p (g i)"),
                    )
                    nc.scalar.dma_start(out_d[s_slot, :, OW - 4 : OW], scl_sb.bitcast(u8))

                v_tiles(range(0, 8))
                for s in range(4):
                    attn_slot(s)
                # hoisted first-half slabs of slots 4-7 (k-tiles 0..7 only);
                # their pt/stats tiles persist into phase 2
                late = {}
                for s in range(4, 8):
                    late[s] = (
                        attn.tile([P, 16, P], bf16, tag=f"ptL{s}", name=f"ptL{s}"),
                        attn.tile([P, 14], f32, tag=f"stL{s}", name=f"stL{s}"),
                    )
                    attn_slot(s, *late[s], si_range=(0, 2))
                x_strips(1, xgb)
                v_tiles(range(8, 16))
                for s in range(4, 8):
                    attn_slot(s, *late[s], si_range=(2, len(slot_slabs(CNT[s]))))

    nc.compile()
    return nc


def _get_program():
    if "nc" not in _CACHE:
        _CACHE["nc"] = _build_program()
    return _CACHE["nc"]


def _get_exec():
    """Jitted shard_map executor with donation of the previous call's
    device-resident outputs (avoids uploading 8.4 MB of host zeros/call)."""
    if "exec" in _CACHE:
        return _CACHE["exec"]
    import jax
    from jax.experimental.shard_map import shard_map
    from jax.sharding import Mesh, PartitionSpec

    from concourse import mybir
    from concourse.bass2jax import (
        _bass_exec_p,
        install_neuronx_cc_hook,
        partition_id_tensor,
    )

    nc = _get_program()
    install_neuronx_cc_hook()

    partition_name = nc.partition_id_tensor.name if nc.partition_id_tensor else None
    in_names, out_names, out_avals = [], [], []
    for alloc in nc.m.functions[0].allocations:
        if not isinstance(alloc, mybir.MemoryLocationSet):
            continue
        name = alloc.memorylocations[0].name
        if alloc.kind == "ExternalInput":
            if name != partition_name:
                in_names.append(name)
        elif alloc.kind == "ExternalOutput":
            assert alloc.tensor_shape is not None and alloc.dtype is not None
            out_names.append(name)
            out_avals.append(
                jax.core.ShapedArray(tuple(alloc.tensor_shape), mybir.dt.np(alloc.dtype))
            )
    n_params = len(in_names)
    n_outs = len(out_names)
    in_names_all = in_names + out_names + ([partition_name] if partition_name else [])

    def _body(*args):
        operands = list(args)
        if partition_name is not None:
            operands.append(partition_id_tensor())
        outs = _bass_exec_p.bind(
            *operands,
            out_avals=tuple(out_avals),
            in_names=tuple(in_names_all),
            out_names=tuple(out_names),
            lowering_input_output_aliases=(),
            sim_require_finite=True,
            sim_require_nnan=True,
            nc=nc,
        )
        return tuple(outs)

    devices = jax.devices()[:NCORES]
    mesh = Mesh(np.asarray(devices), ("core",))
    donate = tuple(range(n_params, n_params + n_outs))
    sharded = jax.jit(
        shard_map(
            _body,
            mesh=mesh,
            in_specs=(PartitionSpec("core"),) * (n_params + n_outs),
            out_specs=(PartitionSpec("core"),) * n_outs,
            check_rep=False,
        ),
        donate_argnums=donate,
        keep_unused=True,
    )
    st = {
        "sharded": sharded,
        "in_names": in_names,
        "out_names": out_names,
        "out_avals": out_avals,
        "prev": None,
    }
    _CACHE["exec"] = st
    return st


def _run_once(maps):
    """One full call: upload inputs, execute on 8 cores, download outputs.

    Returns a per-core list of {name: np.ndarray} like run_bass_kernel_spmd.
    """
    st = _get_exec()
    in_names, out_names, out_avals = st["in_names"], st["out_names"], st["out_avals"]
    concat_in = [
        np.concatenate([np.asarray(m[n]) for m in maps], axis=0) for n in in_names
    ]
    don = st["prev"]
    if don is None:
        don = [
            np.zeros((NCORES * a.shape[0], *a.shape[1:]), a.dtype) for a in out_avals
        ]
    st["prev"] = None
    out_arrs = st["sharded"](*concat_in, *don)
    for o in out_arrs:
        o.copy_to_host_async()
    host = [np.asarray(o) for o in out_arrs]
    st["prev"] = list(out_arrs)
    return [
        {
            name: host[i].reshape(NCORES, *out_avals[i].shape)[c]
            for i, name in enumerate(out_names)
        }
        for c in range(NCORES)
    ]


def _pack_x10(xqt, s):
    """int10-pack [E, 1024] f32 (global power-of-2 scale s) into the
    [E, 1280] u8 wire layout: per 512-col group, 512 int8 high bytes
    (x10 >> 2) then 128 packed low bytes
    (byte j = lo[j] | lo[j+128]<<2 | lo[j+256]<<4 | lo[j+384]<<6)."""
    x10 = np.round(xqt / s).astype(np.int16)
    hi = (x10 >> 2).astype(np.int8)
    lo = (x10 & 3).astype(np.uint8)
    xp = np.empty((E, XW), np.uint8)
    for g in range(2):
        c0 = g * 512
        xp[:, g * GB : g * GB + KSL] = hi[:, c0 : c0 + 512].view(np.uint8)
        xp[:, g * GB + KSL : (g + 1) * GB] = (
            lo[:, c0 : c0 + 128]
            | (lo[:, c0 + 128 : c0 + 256] << 2)
            | (lo[:, c0 + 256 : c0 + 384] << 4)
            | (lo[:, c0 + 384 : c0 + 512] << 6)
        )
    return xp


def _in_maps(embeddings, Wq, Wk, Wv):
    wm = Wq.astype(np.float32) @ Wk.T.astype(np.float32)
    # per-row int8 quantization of wm; scales ride in thr cols 1..8
    s = np.abs(wm).max(axis=1) / 127.0  # [E]
    wm_i8 = np.clip(np.round(wm / s[:, None]), -127, 127).astype(np.int8)
    # per-row int8 quantization of Wv; scales in thr cols 10..17
    sv = np.abs(Wv).max(axis=1).astype(np.float32) / 127.0  # [E]
    wv_i8 = np.clip(np.round(Wv / sv[:, None]), -127, 127).astype(np.int8)
    shard = E // NCORES
    scl_cols = s.reshape(NCORES, P).T.astype(np.float32)  # [ci, co]
    sclv_cols = sv.reshape(NCORES, P).T.astype(np.float32)  # [ci, co]
    # global power-of-2 X scale: one bf16 rounding per dequantized element
    sx = float(2.0 ** np.ceil(np.log2(np.abs(embeddings).max() / 511.0)))
    maps = []
    for c in range(NCORES):
        b, g = divmod(c, 2)
        Xb = embeddings[b]
        xq = np.concatenate([Xb[P * t : P * (t + 1)] for t in TILES[g]], axis=0)
        thr = np.empty((P, 20), np.float32)
        thr[:, 0] = 128.0 * g
        thr[:, 1:9] = scl_cols
        thr[:, 9] = sx
        thr[:, 10:18] = sclv_cols
        thr[:, 18] = 4.0 * sx
        thr[:, 19] = -4.0 * sx
        maps.append(
            {
                "xp": _pack_x10(np.ascontiguousarray(xq.T).astype(np.float32), sx),
                "wmsh": np.ascontiguousarray(wm_i8[c * shard : (c + 1) * shard]),
                "wvsh": np.ascontiguousarray(wv_i8[c * shard : (c + 1) * shard]),
                "thr": thr,
            }
        )
    return maps


_U7_W = np.array([64, 32, 16, 8, 4, 2, 1], np.uint16)


def _gather_out(results):
    out = np.empty((B, S, D), np.float32)
    for c in range(NCORES):
        b, g = divmod(c, 2)
        blob = np.asarray(results[c]["out"])  # [8, P, 900] u8
        o7 = blob[:, :, : D // 8 * 7]
        scl = (
            np.ascontiguousarray(blob[:, :, D // 8 * 7 :])
            .view(np.float32)
            .astype(np.float32)
        )  # [8, P, 1] = rowmax/63
        bits = np.unpackbits(np.ascontiguousarray(o7).reshape(-1, 7), axis=1)
        u = bits.reshape(-1, 8, 7).astype(np.uint16) @ _U7_W  # [N, 8]
        oc = u.reshape(8, P, D).astype(np.float32) - 64.0
        for s_slot, t in enumerate(TILES[g]):
            out[b, P * t : P * (t + 1), :] = oc[s_slot] * scl[s_slot]
    return out


def _run(embeddings, Wq, Wk, Wv):
    maps = _in_maps(embeddings, Wq, Wk, Wv)
    res = _run_once(maps)
    return _gather_out(res), res


def kernel(embeddings, Wq, Wk, Wv):
    embeddings = np.ascontiguousarray(np.asarray(embeddings, dtype=np.float32))
    Wq = np.ascontiguousarray(np.asarray(Wq, dtype=np.float32))
    Wk = np.ascontiguousarray(np.asarray(Wk, dtype=np.float32))
    Wv = np.ascontiguousarray(np.asarray(Wv, dtype=np.float32))
    out, _ = _run(embeddings, Wq, Wk, Wv)
    return out
